# revision 1
# baseline (speedup 1.0000x reference)
"""Trainium2 Bass kernel for blockwise 8x8 DCT layer.

Reference op (per 8x8 block X of each [512,512] image):
    out_block = (D @ X @ D.T).T  =  D @ X.T @ D.T
i.e. out[8i+a, 8j+b] = sum_{k,l} D[a,l] * D[b,k] * x[8i+k, 8j+l]

Strategy (per core, pure data-parallel over batch):
  Each [512,512] image is processed as 4 tiles of [128, 512].  The tile
  layout is chosen so that both DCT passes are dense 128x128 block-diagonal
  matmuls with a single stationary weight matrix W = kron(I16, D.T), and the
  partition<->free exchange needed between the two passes (and for the final
  intra-block transpose) is done by the DVE 32x32 stream transpose plus two
  free-axis permutations folded into the PSUM->SBUF copies.

  Tile-in layout  (host pre-arranged): P=(i2,j2,k), F=(i1,j4,l)
      where image row = 128*I + 32*i2 + 8*i1 + k, col = 128*j2 + 8*j4 + l
  MM1   : contract k -> b          (W.T @ tile)
  evac1 : F (i1,j4,l) -> (j4,i1,l)
  T32   : swap P-intra(j2,b) <-> F-intra(i1,l)   [DVE 32x32 transpose]
  MM2   : contract l -> a          (same W)
  evac2 : F (j4,j2,b) -> (j2,j4,b)
  Result: P = 32*i2+8*i1+a = natural row within the 128-row strip,
          F = 128*j2+8*j4+b = natural column -> contiguous DMA out.
"""

import math
import numpy as np

import concourse.bass as bass
import concourse.tile as tile
from concourse import bacc, mybir
from concourse.bass_utils import run_bass_kernel_spmd

N_CORES = 8
B, C, H, W_IMG = 32, 3, 512, 512
IMGS_PER_CORE = (B // N_CORES) * C  # 12
F32 = mybir.dt.float32


def _dct_basis_np(p=8):
    u = np.arange(p)[:, None]
    x = np.arange(p)[None, :]
    cu = np.where(u == 0, 1.0 / math.sqrt(p), math.sqrt(2.0 / p))
    return (cu * np.cos((2 * x + 1) * u * np.pi / (2 * p))).astype(np.float32)


def _build_nc(n_img, repeat=1):
    nc = bacc.Bacc("TRN2", target_bir_lowering=False, debug=False)
    x_d = nc.dram_tensor("x", [n_img, 4, 128, 512], F32, kind="ExternalInput")
    w_d = nc.dram_tensor("w", [128, 128], F32, kind="ExternalInput")
    y_d = nc.dram_tensor("y", [n_img, 512, 512], F32, kind="ExternalOutput")

    with tile.TileContext(nc) as tc:
        with (
            tc.tile_pool(name="wpool", bufs=1) as wpool,
            tc.tile_pool(name="xin", bufs=4) as xin_pool,
            tc.tile_pool(name="s1", bufs=4) as s1_pool,
            tc.tile_pool(name="s2", bufs=4) as s2_pool,
            tc.tile_pool(name="s3", bufs=4) as s3_pool,
            tc.tile_pool(name="ps1", bufs=3, space="PSUM") as ps1_pool,
            tc.tile_pool(name="ps2", bufs=3, space="PSUM") as ps2_pool,
        ):
            w_t = wpool.tile([128, 128], F32)
            nc.sync.dma_start(w_t[:], w_d[:])

            for img in range(n_img * repeat):
                img = img % n_img
                for I in range(4):
                    xt = xin_pool.tile([128, 512], F32)
                    nc.sync.dma_start(xt[:], x_d[img, I])

                    ps1 = ps1_pool.tile([128, 512], F32)
                    nc.tensor.matmul(ps1[:], w_t[:], xt[:], start=True, stop=True)

                    s1 = s1_pool.tile([128, 512], F32)
                    nc.scalar.copy(
                        s1[:].rearrange("p (j4 i1 l) -> p j4 i1 l", j4=16, i1=4, l=8),
                        ps1[:].rearrange("p (i1 j4 l) -> p j4 i1 l", i1=4, j4=16, l=8),
                    )

                    s2 = s2_pool.tile([128, 512], F32)
                    nc.vector.transpose(s2[:], s1[:])

                    ps2 = ps2_pool.tile([128, 512], F32)
                    nc.tensor.matmul(ps2[:], w_t[:], s2[:], start=True, stop=True)

                    s3 = s3_pool.tile([128, 512], F32)
                    nc.vector.tensor_copy(
                        s3[:].rearrange("p (j2 j4 b) -> p j2 j4 b", j2=4, j4=16, b=8),
                        ps2[:].rearrange("p (j4 j2 b) -> p j2 j4 b", j4=16, j2=4, b=8),
                    )

                    nc.sync.dma_start(y_d[img, 128 * I : 128 * (I + 1), :], s3[:])

    nc.compile()
    return nc


_NC_CACHE = {}
LAST_RESULTS = None
LAST_IN_MAPS = None


def _get_nc(n_img):
    if n_img not in _NC_CACHE:
        _NC_CACHE[n_img] = _build_nc(n_img)
    return _NC_CACHE[n_img]


def _host_rearrange(xc):
    """[n_img, 512, 512] -> [n_img, 4, 128, 512] device tile layout."""
    n = xc.shape[0]
    t = xc.reshape(n, 4, 4, 4, 8, 4, 16, 8)  # (img, I, i2, i1, k, j2, j4, l)
    t = t.transpose(0, 1, 2, 5, 4, 3, 6, 7)  # (img, I, i2, j2, k, i1, j4, l)
    return np.ascontiguousarray(t).reshape(n, 4, 128, 512)


def kernel(x, dct_basis=None, **_unused):
    x = np.asarray(x, dtype=np.float32)
    if dct_basis is None:
        D = _dct_basis_np()
    else:
        D = np.asarray(dct_basis, dtype=np.float32)
    Wm = np.kron(np.eye(16, dtype=np.float32), D.T).astype(np.float32)
    Wm = np.ascontiguousarray(Wm)

    bsz = x.shape[0]
    per_core = bsz // N_CORES
    n_img = per_core * x.shape[1]

    nc = _get_nc(n_img)

    in_maps = []
    for c in range(N_CORES):
        xc = x[c * per_core : (c + 1) * per_core].reshape(n_img, H, W_IMG)
        in_maps.append({"x": _host_rearrange(xc), "w": Wm})

    global LAST_RESULTS, LAST_IN_MAPS
    LAST_IN_MAPS = in_maps
    res = run_bass_kernel_spmd(nc, in_maps, list(range(N_CORES)))
    LAST_RESULTS = res

    out = np.empty((bsz, x.shape[1], H, W_IMG), dtype=np.float32)
    for c in range(N_CORES):
        out[c * per_core : (c + 1) * per_core] = res.results[c]["y"].reshape(
            per_core, x.shape[1], H, W_IMG
        )
    return out


if __name__ == "__main__":
    xs = np.random.randn(B, C, H, W_IMG).astype(np.float32)
    y = kernel(xs)
    print("kernel ran, output shape", y.shape)



# revision 2
# speedup vs baseline: 1.3973x; 1.3973x over previous
"""Trainium2 Bass kernel for blockwise 8x8 DCT layer (bf16 pipeline).

Reference op (per 8x8 block X of each [512,512] image):
    out_block = (D @ X @ D.T).T
i.e. stage1  Y1[b,l] = sum_k D[b,k] X[k,l]
     stage2  out[a,b] = sum_l D[a,l] Y1[b,l]

Strategy (per core, pure data-parallel over batch, all device I/O in bf16 —
the 2e-2 tolerance admits bf16 with ~3x margin; this halves HBM traffic,
which is the roofline):

  Image rows: 512 = 128*I + 32*i2 + 8*i1 + k     (strip I, block-row (i2,i1))
  Image cols: 512 = 128*j2 + 8*j4 + l            (block-col (j2,j4))

  Host input layout  x_d[128, n*2048] bf16: P=(i2,j2,k), F=(n,I,j4,i1,l)
  MM1   : W=kron(I16,D.T) stationary, contract k -> b        (PSUM f32)
  evac1 : ACT plain copy + cast -> bf16                       (no permute)
  T32   : DVE 32x32 stream transpose: P(j2,b)<->F(i1,l)
          -> P=(i2,i1,l), F=(j4,j2,b)
  MM2   : same W, contract l -> a                             (PSUM f32)
  evac2 : DVE plain copy + cast -> bf16
  Out layout y_d[128, n*2048] bf16: P=(i2,i1,a)=row-in-strip, F=(n,I,j4,j2,b)
  Host unshards the (j4,j2) column interleave + strips in numpy.

  DMAs move 2 images (1 MiB) per transfer, contiguous 8KB per partition.
"""

import math
import numpy as np
import ml_dtypes

import concourse.bass as bass
import concourse.tile as tile
from concourse import bacc, mybir
from concourse.bass_utils import run_bass_kernel_spmd

N_CORES = 8
B, C, H, W_IMG = 32, 3, 512, 512
IMGS_PER_CORE = (B // N_CORES) * C  # 12
F32 = mybir.dt.float32
BF16 = mybir.dt.bfloat16
NP_BF16 = ml_dtypes.bfloat16
CHUNK_IMGS = 2  # images per DMA transfer (1 MiB in bf16)


def _dct_basis_np(p=8):
    u = np.arange(p)[:, None]
    x = np.arange(p)[None, :]
    cu = np.where(u == 0, 1.0 / math.sqrt(p), math.sqrt(2.0 / p))
    return (cu * np.cos((2 * x + 1) * u * np.pi / (2 * p))).astype(np.float32)


def _build_nc(n_img, repeat=1):
    nc = bacc.Bacc("TRN2", target_bir_lowering=False, debug=False)
    fw = n_img * 2048
    x_d = nc.dram_tensor("x", [128, fw], BF16, kind="ExternalInput")
    w_d = nc.dram_tensor("w", [128, 128], BF16, kind="ExternalInput")
    y_d = nc.dram_tensor("y", [128, fw], BF16, kind="ExternalOutput")

    n_chunks = n_img // CHUNK_IMGS
    cw = CHUNK_IMGS * 2048  # free width per chunk

    with tile.TileContext(nc) as tc:
        with (
            tc.tile_pool(name="wpool", bufs=1) as wpool,
            tc.tile_pool(name="xin", bufs=3) as xin_pool,
            tc.tile_pool(name="s1", bufs=4) as s1_pool,
            tc.tile_pool(name="s2", bufs=4) as s2_pool,
            tc.tile_pool(name="yout", bufs=3) as yout_pool,
            tc.tile_pool(name="ps1", bufs=3, space="PSUM") as ps1_pool,
            tc.tile_pool(name="ps2", bufs=3, space="PSUM") as ps2_pool,
        ):
            w_t = wpool.tile([128, 128], BF16)
            nc.sync.dma_start(w_t[:], w_d[:])

            for it in range(n_chunks * repeat):
                c = it % n_chunks
                xt = xin_pool.tile([128, cw], BF16)
                nc.sync.dma_start(xt[:], x_d[:, c * cw : (c + 1) * cw])

                yt = yout_pool.tile([128, cw], BF16)
                for s in range(cw // 512):
                    xsl = xt[:, 512 * s : 512 * (s + 1)]

                    ps1 = ps1_pool.tile([128, 512], F32)
                    nc.tensor.matmul(ps1[:], w_t[:], xsl, start=True, stop=True)

                    s1 = s1_pool.tile([128, 512], BF16)
                    nc.scalar.copy(s1[:], ps1[:])

                    s2 = s2_pool.tile([128, 512], BF16)
                    nc.vector.transpose(s2[:], s1[:])

                    ps2 = ps2_pool.tile([128, 512], F32)
                    nc.tensor.matmul(ps2[:], w_t[:], s2[:], start=True, stop=True)

                    nc.vector.tensor_copy(yt[:, 512 * s : 512 * (s + 1)], ps2[:])

                nc.sync.dma_start(y_d[:, c * cw : (c + 1) * cw], yt[:])

    nc.compile()
    return nc


_NC_CACHE = {}
LAST_RESULTS = None
LAST_IN_MAPS = None


def _get_nc(n_img):
    if n_img not in _NC_CACHE:
        _NC_CACHE[n_img] = _build_nc(n_img)
    return _NC_CACHE[n_img]


def _host_rearrange(xc):
    """[n, 512, 512] f32 -> [128, n*2048] bf16 device layout."""
    n = xc.shape[0]
    t = xc.astype(NP_BF16).reshape(n, 4, 4, 4, 8, 4, 16, 8)  # (n,I,i2,i1,k,j2,j4,l)
    t = t.transpose(2, 5, 4, 0, 1, 6, 3, 7)  # (i2,j2,k,n,I,j4,i1,l)
    return np.ascontiguousarray(t).reshape(128, n * 2048)


def _host_unshard(yc, n):
    """[128, n*2048] bf16 -> [n, 512, 512] f32."""
    t = yc.reshape(128, n, 4, 16, 4, 8)  # (r,n,I,j4,j2,b)
    t = t.transpose(1, 2, 0, 4, 3, 5)  # (n,I,r,j2,j4,b)
    return np.ascontiguousarray(t).astype(np.float32).reshape(n, 512, 512)


def kernel(x, dct_basis=None, **_unused):
    x = np.asarray(x, dtype=np.float32)
    if dct_basis is None:
        D = _dct_basis_np()
    else:
        D = np.asarray(dct_basis, dtype=np.float32)
    Wm = np.ascontiguousarray(
        np.kron(np.eye(16, dtype=np.float32), D.T)
    ).astype(NP_BF16)

    bsz = x.shape[0]
    per_core = bsz // N_CORES
    n_img = per_core * x.shape[1]

    nc = _get_nc(n_img)

    in_maps = []
    for c in range(N_CORES):
        xc = x[c * per_core : (c + 1) * per_core].reshape(n_img, H, W_IMG)
        in_maps.append({"x": _host_rearrange(xc), "w": Wm})

    global LAST_RESULTS, LAST_IN_MAPS
    LAST_IN_MAPS = in_maps
    res = run_bass_kernel_spmd(nc, in_maps, list(range(N_CORES)))
    LAST_RESULTS = res

    out = np.empty((bsz, x.shape[1], H, W_IMG), dtype=np.float32)
    for c in range(N_CORES):
        out[c * per_core : (c + 1) * per_core] = _host_unshard(
            res.results[c]["y"], n_img
        ).reshape(per_core, x.shape[1], H, W_IMG)
    return out


if __name__ == "__main__":
    xs = np.random.randn(B, C, H, W_IMG).astype(np.float32)
    y = kernel(xs)
    print("kernel ran, output shape", y.shape)
